# revision 6
# baseline (speedup 1.0000x reference)
"""Alignment generator (length regulator) on 8 TRN2 NeuronCores.

out[b, f, j] = 1.0  iff  starts[b,j] <= f < ends[b,j]  (ends = cumsum(dur))

Each output row out[b, f, :] is one-hot at token_id[b, f] =
searchsorted(ends[b], f, side='right') (or all-zero when f >= total frames).
The host computes token_id from the tiny [32, 512] duration input; each core
then generates its 4-row slab of the ~256MB output with one DVE
tensor_scalar(is_equal) per [128-frame x 512-token] tile (fp32 tensor_scalar
runs in 2x_2P mode) and streams it out in ~2MB HWDGE DMAs.

Raw Bass (no Tile): this walrus build only allows a single sync-wait per
compute/DMA instruction, so all synchronization is explicit standalone
wait_ge with a ring of NBUF buffers, one completion semaphore per buffer
slot (per-slot sems make "slot's previous DMA fully drained" provable from
a 16*m threshold).

Sharding: pure data parallelism, batch dim 32 -> 4 rows per core.
"""

import math
from contextlib import ExitStack

import numpy as np

import concourse.bass as bass
import concourse.mybir as mybir
from concourse.bass_utils import run_bass_kernel_spmd

N_CORES = 8
B = 32          # batch
T = 512         # tokens
P = 128         # SBUF partitions (frames per tile)
GROUP = 8       # frame-tiles per output DMA (8*128*512*4B = 2 MiB)
NBUF = 4        # output buffer ring slots

_nc_cache: dict[tuple[int, int], bass.Bass] = {}


def _build(m_frames: int, b_loc: int) -> bass.Bass:
    """Per-core Bass graph writing a [b_loc, m_pad, T] padded output slab."""
    ntiles = math.ceil(m_frames / P)
    m_pad = ntiles * P

    # rounds: (row, first_tile, n_tiles_in_chunk)
    rounds = []
    for b in range(b_loc):
        for g0 in range(0, ntiles, GROUP):
            rounds.append((b, g0, min(GROUP, ntiles - g0)))
    n_rounds = len(rounds)

    nc = bass.Bass()
    # input column layout: [0:T) = iota row J (J[p,j] = j), [T:) = token ids
    # tid_sb[p, T + b*ntiles + t] = token_id[b, t*128 + p]
    tid = nc.declare_dram_parameter(
        "tid", [P, T + b_loc * ntiles], mybir.dt.float32, isOutput=False
    )
    out = nc.declare_dram_parameter(
        "out", [b_loc, m_pad, T], mybir.dt.float32, isOutput=True
    )

    with ExitStack() as ctx:
        sb = ctx.enter_context(
            nc.sbuf_tensor("sb", [P, T + b_loc * ntiles], mybir.dt.float32)
        )
        bufs = [
            ctx.enter_context(
                nc.sbuf_tensor(f"buf{s}", [P, GROUP * T], mybir.dt.float32)
            )
            for s in range(NBUF)
        ]
        in_sem = ctx.enter_context(nc.semaphore("in_sem"))
        c_sem = ctx.enter_context(nc.semaphore("c_sem"))
        d_sems = [ctx.enter_context(nc.semaphore(f"d_sem{s}")) for s in range(NBUF)]
        block = ctx.enter_context(nc.Block())

        @block.sync
        def _(sync):
            sync.dma_start(out=sb[:, :], in_=tid[:, :]).then_inc(in_sem, 16)
            for r, (b, g0, g) in enumerate(rounds):
                sync.wait_ge(c_sem, r + 1)
                dview = out[b, g0 * P : (g0 + g) * P, :].rearrange(
                    "(k p) t -> p k t", p=P
                )
                sync.dma_start(
                    out=dview,
                    in_=bufs[r % NBUF][:, : g * T].rearrange("p (k t) -> p k t", t=T),
                ).then_inc(d_sems[r % NBUF], 16)
            # all output bytes landed before the NEFF may finish
            for s in range(NBUF):
                uses = len(range(s, n_rounds, NBUF))
                if uses:
                    sync.wait_ge(d_sems[s], 16 * uses)

        @block.vector
        def _(vector):
            vector.wait_ge(in_sem, 16)
            for r, (b, g0, g) in enumerate(rounds):
                s = r % NBUF
                if r >= NBUF:
                    # slot's previous DMA (round r-NBUF, its (r//NBUF)-th use)
                    # fully drained
                    vector.wait_ge(d_sems[s], 16 * (r // NBUF))
                last = None
                for k in range(g):
                    col = T + b * ntiles + g0 + k
                    last = nc.vector.tensor_scalar(
                        out=bufs[s][:, k * T : (k + 1) * T],
                        in0=sb[:, 0:T],
                        scalar1=sb[:, col : col + 1],
                        scalar2=None,
                        op0=mybir.AluOpType.is_equal,
                    )
                last.then_inc(c_sem, 1)

    return nc


def _token_ids(dur: np.ndarray, m_pad: int) -> np.ndarray:
    """tid[b, f] = index of the token whose frame interval contains f,
    or T (out of range -> all-zero row) when no token covers frame f."""
    ends = np.cumsum(dur.astype(np.int64), axis=1)
    frames = np.arange(m_pad, dtype=np.int64)
    tid = np.empty((dur.shape[0], m_pad), dtype=np.float32)
    for b in range(dur.shape[0]):
        tid[b] = np.searchsorted(ends[b], frames, side="right")
    return tid


def _prepare(duration_predictor_output: np.ndarray, max_frames):
    """Host-side prep: token ids, per-core input maps, cached Bass graph."""
    dur = np.asarray(duration_predictor_output)
    m_frames = int(max_frames)
    b_loc = B // N_CORES
    ntiles = math.ceil(m_frames / P)
    m_pad = ntiles * P

    tid = _token_ids(dur, m_pad)  # [B, m_pad] float32
    iota_row = np.broadcast_to(np.arange(T, dtype=np.float32), (P, T))

    key = (m_frames, b_loc)
    nc = _nc_cache.get(key)
    if nc is None:
        nc = _build(m_frames, b_loc)
        _nc_cache[key] = nc

    in_maps = []
    for i in range(N_CORES):
        rows = tid[i * b_loc : (i + 1) * b_loc]              # [b_loc, m_pad]
        # [b_loc, ntiles, P] -> [P, b_loc, ntiles] -> [P, b_loc*ntiles]
        tid_t = rows.reshape(b_loc, ntiles, P).transpose(2, 0, 1).reshape(P, -1)
        in_maps.append(
            {"tid": np.ascontiguousarray(np.concatenate([iota_row, tid_t], axis=1))}
        )
    return nc, in_maps


def kernel(duration_predictor_output: np.ndarray, max_frames) -> np.ndarray:
    dur = np.asarray(duration_predictor_output)
    m_frames = int(max_frames)
    if m_frames <= 0:
        return np.zeros((dur.shape[0], 0, dur.shape[1]), dtype=np.float32)

    nc, in_maps = _prepare(dur, m_frames)
    res = run_bass_kernel_spmd(nc, in_maps, core_ids=list(range(N_CORES)))
    full = np.concatenate(
        [res.results[i]["out"] for i in range(N_CORES)], axis=0
    )
    return np.ascontiguousarray(full[:, :m_frames, :])


# revision 8
# speedup vs baseline: 1.0161x; 1.0161x over previous
"""Alignment generator (length regulator) on 8 TRN2 NeuronCores.

out[b, f, j] = 1.0  iff  starts[b,j] <= f < ends[b,j]  (ends = cumsum(dur))

Each output row out[b, f, :] is one-hot at token_id[b, f] =
searchsorted(ends[b], f, side='right') (or all-zero when f >= total frames).
The host computes token_id from the tiny [32, 512] duration input; each core
then generates its 4-row slab of the ~256MB output with one DVE
tensor_scalar(is_equal) per [128-frame x 512-token] tile (fp32 tensor_scalar
runs in 2x_2P mode) and streams it out in ~2MB HWDGE DMAs.

Raw Bass (no Tile): this walrus build only allows a single sync-wait per
compute/DMA instruction, so all synchronization is explicit standalone
wait_ge with a ring of NBUF buffers, one completion semaphore per buffer
slot (per-slot sems make "slot's previous DMA fully drained" provable from
a 16*m threshold).

Sharding: pure data parallelism, batch dim 32 -> 4 rows per core.
"""

import math
from contextlib import ExitStack

import numpy as np

import concourse.bass as bass
import concourse.mybir as mybir
from concourse.bass_utils import run_bass_kernel_spmd

N_CORES = 8
B = 32          # batch
T = 512         # tokens
P = 128         # SBUF partitions (frames per tile)
GROUP = 8       # frame-tiles per output DMA (8*128*512*4B = 2 MiB)
NBUF = 4        # output buffer ring slots

_nc_cache: dict[tuple[int, int], bass.Bass] = {}


def _build(m_frames: int, b_loc: int) -> bass.Bass:
    """Per-core Bass graph writing a [b_loc, m_pad, T] padded output slab."""
    ntiles = math.ceil(m_frames / P)
    m_pad = ntiles * P

    # rounds: (row, first_tile, n_tiles_in_chunk). Ramp the first row's
    # chunks (1,1,2,4,...) so the first output DMA is issued as soon as
    # possible after the input lands -- the DMA stream is the bottleneck
    # and every ns it starts earlier is a ns off the kernel.
    rounds = []
    for b in range(b_loc):
        g0 = 0
        ramp = [1, 1, 2, 4] if b == 0 else []
        for g in ramp:
            if g0 + g > ntiles:
                break
            rounds.append((b, g0, g))
            g0 += g
        while g0 < ntiles:
            g = min(GROUP, ntiles - g0)
            rounds.append((b, g0, g))
            g0 += g
    n_rounds = len(rounds)

    nc = bass.Bass()
    # input column layout: [0:T) = iota row J (J[p,j] = j), [T:) = token ids
    # tid_sb[p, T + b*ntiles + t] = token_id[b, t*128 + p]
    tid = nc.declare_dram_parameter(
        "tid", [P, T + b_loc * ntiles], mybir.dt.float32, isOutput=False
    )
    out = nc.declare_dram_parameter(
        "out", [b_loc, m_pad, T], mybir.dt.float32, isOutput=True
    )

    with ExitStack() as ctx:
        sb = ctx.enter_context(
            nc.sbuf_tensor("sb", [P, T + b_loc * ntiles], mybir.dt.float32)
        )
        bufs = [
            ctx.enter_context(
                nc.sbuf_tensor(f"buf{s}", [P, GROUP * T], mybir.dt.float32)
            )
            for s in range(NBUF)
        ]
        in_sem = ctx.enter_context(nc.semaphore("in_sem"))
        c_sem = ctx.enter_context(nc.semaphore("c_sem"))
        d_sems = [ctx.enter_context(nc.semaphore(f"d_sem{s}")) for s in range(NBUF)]
        block = ctx.enter_context(nc.Block())

        @block.gpsimd
        def _(gpsimd):
            # SWDGE input load: the GpSimd sequencer clears the NEFF
            # preamble earliest, so the token ids land sooner than an
            # SP-issued HWDGE load would.
            gpsimd.dma_start(out=sb[:, :], in_=tid[:, :]).then_inc(in_sem, 16)

        @block.sync
        def _(sync):
            for r, (b, g0, g) in enumerate(rounds):
                sync.wait_ge(c_sem, r + 1)
                dview = out[b, g0 * P : (g0 + g) * P, :].rearrange(
                    "(k p) t -> p k t", p=P
                )
                sync.dma_start(
                    out=dview,
                    in_=bufs[r % NBUF][:, : g * T].rearrange("p (k t) -> p k t", t=T),
                ).then_inc(d_sems[r % NBUF], 16)
            # all output bytes landed before the NEFF may finish
            for s in range(NBUF):
                uses = len(range(s, n_rounds, NBUF))
                if uses:
                    sync.wait_ge(d_sems[s], 16 * uses)

        @block.vector
        def _(vector):
            vector.wait_ge(in_sem, 16)
            for r, (b, g0, g) in enumerate(rounds):
                s = r % NBUF
                if r >= NBUF:
                    # slot's previous DMA (round r-NBUF, its (r//NBUF)-th use)
                    # fully drained
                    vector.wait_ge(d_sems[s], 16 * (r // NBUF))
                last = None
                for k in range(g):
                    col = T + b * ntiles + g0 + k
                    last = nc.vector.tensor_scalar(
                        out=bufs[s][:, k * T : (k + 1) * T],
                        in0=sb[:, 0:T],
                        scalar1=sb[:, col : col + 1],
                        scalar2=None,
                        op0=mybir.AluOpType.is_equal,
                    )
                last.then_inc(c_sem, 1)

    return nc


def _token_ids(dur: np.ndarray, m_pad: int) -> np.ndarray:
    """tid[b, f] = index of the token whose frame interval contains f,
    or T (out of range -> all-zero row) when no token covers frame f."""
    ends = np.cumsum(dur.astype(np.int64), axis=1)
    frames = np.arange(m_pad, dtype=np.int64)
    tid = np.empty((dur.shape[0], m_pad), dtype=np.float32)
    for b in range(dur.shape[0]):
        tid[b] = np.searchsorted(ends[b], frames, side="right")
    return tid


def _prepare(duration_predictor_output: np.ndarray, max_frames):
    """Host-side prep: token ids, per-core input maps, cached Bass graph."""
    dur = np.asarray(duration_predictor_output)
    m_frames = int(max_frames)
    b_loc = B // N_CORES
    ntiles = math.ceil(m_frames / P)
    m_pad = ntiles * P

    tid = _token_ids(dur, m_pad)  # [B, m_pad] float32
    iota_row = np.broadcast_to(np.arange(T, dtype=np.float32), (P, T))

    key = (m_frames, b_loc)
    nc = _nc_cache.get(key)
    if nc is None:
        nc = _build(m_frames, b_loc)
        _nc_cache[key] = nc

    in_maps = []
    for i in range(N_CORES):
        rows = tid[i * b_loc : (i + 1) * b_loc]              # [b_loc, m_pad]
        # [b_loc, ntiles, P] -> [P, b_loc, ntiles] -> [P, b_loc*ntiles]
        tid_t = rows.reshape(b_loc, ntiles, P).transpose(2, 0, 1).reshape(P, -1)
        in_maps.append(
            {"tid": np.ascontiguousarray(np.concatenate([iota_row, tid_t], axis=1))}
        )
    return nc, in_maps


def kernel(duration_predictor_output: np.ndarray, max_frames) -> np.ndarray:
    dur = np.asarray(duration_predictor_output)
    m_frames = int(max_frames)
    if m_frames <= 0:
        return np.zeros((dur.shape[0], 0, dur.shape[1]), dtype=np.float32)

    nc, in_maps = _prepare(dur, m_frames)
    res = run_bass_kernel_spmd(nc, in_maps, core_ids=list(range(N_CORES)))
    full = np.concatenate(
        [res.results[i]["out"] for i in range(N_CORES)], axis=0
    )
    return np.ascontiguousarray(full[:, :m_frames, :])


# revision 9
# speedup vs baseline: 1.0225x; 1.0063x over previous
"""Alignment generator (length regulator) on 8 TRN2 NeuronCores.

out[b, f, j] = 1.0  iff  starts[b,j] <= f < ends[b,j]  (ends = cumsum(dur))

Each output row out[b, f, :] is one-hot at token_id[b, f] =
searchsorted(ends[b], f, side='right') (or all-zero when f >= total frames).
The host computes token_id from the tiny [32, 512] duration input; each core
then generates its 4-row slab of the ~256MB output with one DVE
tensor_scalar(is_equal) per [128-frame x 512-token] tile (fp32 tensor_scalar
runs in 2x_2P mode) and streams it out in ~2MB HWDGE DMAs.

Raw Bass (no Tile): this walrus build only allows a single sync-wait per
compute/DMA instruction, so all synchronization is explicit standalone
wait_ge with a ring of NBUF buffers, one completion semaphore per buffer
slot (per-slot sems make "slot's previous DMA fully drained" provable from
a 16*m threshold).

Sharding: pure data parallelism, batch dim 32 -> 4 rows per core.
"""

import math
from contextlib import ExitStack

import numpy as np

import concourse.bass as bass
import concourse.mybir as mybir
from concourse.bass_utils import run_bass_kernel_spmd

N_CORES = 8
B = 32          # batch
T = 512         # tokens
P = 128         # SBUF partitions (frames per tile)
GROUP = 8       # frame-tiles per output DMA (8*128*512*4B = 2 MiB)
NBUF = 4        # output buffer ring slots

_nc_cache: dict[tuple[int, int], bass.Bass] = {}


def _build(m_frames: int, b_loc: int) -> bass.Bass:
    """Per-core Bass graph writing a [b_loc, m_pad, T] padded output slab."""
    ntiles = math.ceil(m_frames / P)
    m_pad = ntiles * P

    # rounds: (row, first_tile, n_tiles_in_chunk). Ramp the first row's
    # chunks (1,1,2,4,...) so the first output DMA is issued as soon as
    # possible after the input lands -- the DMA stream is the bottleneck
    # and every ns it starts earlier is a ns off the kernel.
    rounds = []
    for b in range(b_loc):
        g0 = 0
        ramp = [1, 1, 2, 4] if b == 0 else []
        for g in ramp:
            if g0 + g > ntiles:
                break
            rounds.append((b, g0, g))
            g0 += g
        while g0 < ntiles:
            g = min(GROUP, ntiles - g0)
            rounds.append((b, g0, g))
            g0 += g
    n_rounds = len(rounds)

    nc = bass.Bass()
    # input column layout: [0:T) = iota row J (J[p,j] = j), [T:) = token ids
    # tid_sb[p, T + b*ntiles + t] = token_id[b, t*128 + p]
    tid = nc.declare_dram_parameter(
        "tid", [P, T + b_loc * ntiles], mybir.dt.float32, isOutput=False
    )
    out = nc.declare_dram_parameter(
        "out", [b_loc, m_pad, T], mybir.dt.float32, isOutput=True
    )

    with ExitStack() as ctx:
        sb = ctx.enter_context(
            nc.sbuf_tensor("sb", [P, T + b_loc * ntiles], mybir.dt.float32)
        )
        bufs = [
            ctx.enter_context(
                nc.sbuf_tensor(f"buf{s}", [P, GROUP * T], mybir.dt.float32)
            )
            for s in range(NBUF)
        ]
        in_sem = ctx.enter_context(nc.semaphore("in_sem"))
        c_sem = ctx.enter_context(nc.semaphore("c_sem"))
        d_sems = [ctx.enter_context(nc.semaphore(f"d_sem{s}")) for s in range(NBUF)]
        block = ctx.enter_context(nc.Block())

        @block.sync
        def _(sync):
            sync.dma_start(out=sb[:, :], in_=tid[:, :]).then_inc(in_sem, 16)
            for r, (b, g0, g) in enumerate(rounds):
                sync.wait_ge(c_sem, r + 1)
                dview = out[b, g0 * P : (g0 + g) * P, :].rearrange(
                    "(k p) t -> p k t", p=P
                )
                sync.dma_start(
                    out=dview,
                    in_=bufs[r % NBUF][:, : g * T].rearrange("p (k t) -> p k t", t=T),
                ).then_inc(d_sems[r % NBUF], 16)
            # all output bytes landed before the NEFF may finish
            for s in range(NBUF):
                uses = len(range(s, n_rounds, NBUF))
                if uses:
                    sync.wait_ge(d_sems[s], 16 * uses)

        @block.vector
        def _(vector):
            vector.wait_ge(in_sem, 16)
            for r, (b, g0, g) in enumerate(rounds):
                s = r % NBUF
                if r >= NBUF:
                    # slot's previous DMA (round r-NBUF, its (r//NBUF)-th use)
                    # fully drained
                    vector.wait_ge(d_sems[s], 16 * (r // NBUF))
                last = None
                for k in range(g):
                    col = T + b * ntiles + g0 + k
                    last = nc.vector.tensor_scalar(
                        out=bufs[s][:, k * T : (k + 1) * T],
                        in0=sb[:, 0:T],
                        scalar1=sb[:, col : col + 1],
                        scalar2=None,
                        op0=mybir.AluOpType.is_equal,
                    )
                last.then_inc(c_sem, 1)

    return nc


def _token_ids(dur: np.ndarray, m_pad: int) -> np.ndarray:
    """tid[b, f] = index of the token whose frame interval contains f,
    or T (out of range -> all-zero row) when no token covers frame f."""
    ends = np.cumsum(dur.astype(np.int64), axis=1)
    frames = np.arange(m_pad, dtype=np.int64)
    tid = np.empty((dur.shape[0], m_pad), dtype=np.float32)
    for b in range(dur.shape[0]):
        tid[b] = np.searchsorted(ends[b], frames, side="right")
    return tid


def _prepare(duration_predictor_output: np.ndarray, max_frames):
    """Host-side prep: token ids, per-core input maps, cached Bass graph."""
    dur = np.asarray(duration_predictor_output)
    m_frames = int(max_frames)
    b_loc = B // N_CORES
    ntiles = math.ceil(m_frames / P)
    m_pad = ntiles * P

    tid = _token_ids(dur, m_pad)  # [B, m_pad] float32
    iota_row = np.broadcast_to(np.arange(T, dtype=np.float32), (P, T))

    key = (m_frames, b_loc)
    nc = _nc_cache.get(key)
    if nc is None:
        nc = _build(m_frames, b_loc)
        _nc_cache[key] = nc

    in_maps = []
    for i in range(N_CORES):
        rows = tid[i * b_loc : (i + 1) * b_loc]              # [b_loc, m_pad]
        # [b_loc, ntiles, P] -> [P, b_loc, ntiles] -> [P, b_loc*ntiles]
        tid_t = rows.reshape(b_loc, ntiles, P).transpose(2, 0, 1).reshape(P, -1)
        in_maps.append(
            {"tid": np.ascontiguousarray(np.concatenate([iota_row, tid_t], axis=1))}
        )
    return nc, in_maps


def kernel(duration_predictor_output: np.ndarray, max_frames) -> np.ndarray:
    dur = np.asarray(duration_predictor_output)
    m_frames = int(max_frames)
    if m_frames <= 0:
        return np.zeros((dur.shape[0], 0, dur.shape[1]), dtype=np.float32)

    nc, in_maps = _prepare(dur, m_frames)
    res = run_bass_kernel_spmd(nc, in_maps, core_ids=list(range(N_CORES)))
    full = np.concatenate(
        [res.results[i]["out"] for i in range(N_CORES)], axis=0
    )
    return np.ascontiguousarray(full[:, :m_frames, :])
